# revision 14
# baseline (speedup 1.0000x reference)
"""Column-sum kernel for Trainium2: out[d] = sum_r x[r, d].

x is [8192, 4096] f32, rows sharded across 8 NeuronCores (1024 rows
each). Per core the shard is loaded as COLUMN blocks: one DMA brings
all 1024 rows of a W-column block into SBUF as 8 side-by-side
[128, W] sub-tiles. DVE folds the 8 sub-tiles into one [128, W]
accumulator (7 in-place adds, hidden under the next block's DMA), the
PE closes that block's ones-matmul partition reduce immediately, and
ACT copies PSUM out. Column blocks keep the reduce pipelined with the
load stream, so after the last DMA only one small block's fold +
matmul remains (the last block is deliberately narrow). fp32 PE
matmuls are half-rate (LOW_HIGH), so keeping PE work to one matmul
per 512 columns total — instead of one per tile — is what makes this
memory-bound instead of PE-bound. Host sums the 8 per-core partials.
"""

import numpy as np

M_CORES = 8
ROWS, D = 8192, 4096
ROWS_PER_CORE = ROWS // M_CORES  # 1024
P = 128
J_TILES = ROWS_PER_CORE // P  # 8 row sub-tiles per column block
BLOCK_W = (640, 640, 640, 640, 640, 640, 256)  # sums to 4096
NCHUNK = 512  # fp32 PSUM bank capacity / max fp32 moving free dim

_nc_cache = None


def _build():
    import concourse.tile as tile
    from concourse import bacc, mybir

    nc = bacc.Bacc(None)
    x = nc.declare_dram_parameter(
        "x", [ROWS_PER_CORE, D], mybir.dt.float32, isOutput=False
    )
    out = nc.declare_dram_parameter("out", [1, D], mybir.dt.float32, isOutput=True)

    x3 = x[:].rearrange("(j p) d -> p j d", p=P)  # [128, 8, 4096]

    with tile.TileContext(nc) as tc:
        with (
            tc.tile_pool(name="xpool", bufs=3) as xpool,
            tc.tile_pool(name="accpool", bufs=2) as accpool,
            tc.tile_pool(name="singles", bufs=1) as singles,
            tc.tile_pool(name="psum", bufs=4, space="PSUM") as psum_pool,
        ):
            ones = singles.tile([P, 1], mybir.dt.float32)
            nc.vector.memset(ones[:], 1.0)

            osb = singles.tile([1, D], mybir.dt.float32)

            col = 0
            for b, W in enumerate(BLOCK_W):
                xt = xpool.tile([P, J_TILES * W], mybir.dt.float32,
                                name=f"xt{b}", tag="xt")
                nc.sync.dma_start(
                    xt[:].rearrange("p (j w) -> p j w", j=J_TILES),
                    x3[:, :, col : col + W],
                )

                acc = accpool.tile([P, W], mybir.dt.float32,
                                   name=f"acc{b}", tag="acc")
                nc.vector.tensor_add(acc[:], xt[:, 0:W], xt[:, W : 2 * W])
                for j in range(2, J_TILES):
                    nc.vector.tensor_add(acc[:], acc[:], xt[:, j * W : (j + 1) * W])

                for s0 in range(0, W, NCHUNK):
                    sw = min(NCHUNK, W - s0)
                    ps = psum_pool.tile([1, NCHUNK], mybir.dt.float32,
                                        name=f"ps{b}_{s0}", tag="ps")
                    nc.tensor.matmul(
                        ps[:1, :sw], ones[:], acc[:, s0 : s0 + sw],
                        start=True, stop=True,
                    )
                    nc.scalar.copy(osb[:, col + s0 : col + s0 + sw], ps[:1, :sw])
                col += W

            nc.sync.dma_start(out[:, :], osb[:])

    nc.compile()
    return nc


def _get_nc():
    global _nc_cache
    if _nc_cache is None:
        _nc_cache = _build()
    return _nc_cache


def _run(x_np: np.ndarray, **run_kwargs):
    from concourse.bass_utils import run_bass_kernel_spmd

    nc = _get_nc()
    shards = np.split(x_np, M_CORES, axis=0)
    in_maps = [{"x": np.ascontiguousarray(s)} for s in shards]
    return run_bass_kernel_spmd(nc, in_maps, list(range(M_CORES)), **run_kwargs)


def kernel(x) -> np.ndarray:
    x_np = np.ascontiguousarray(np.asarray(x), dtype=np.float32)
    assert x_np.shape == (ROWS, D), x_np.shape
    res = _run(x_np)
    partials = np.stack([r["out"][0] for r in res.results])
    return partials.sum(axis=0, dtype=np.float32)
